# revision 1
# baseline (speedup 1.0000x reference)
"""Trainium2 Bass kernel for top-2 MoE (nn_MoE_2113123910117).

Strategy (expert-parallel, per sharding hint):
  - Host: router logits -> softmax -> top-2 -> normalized combine weights;
    dispatch tokens to 8 expert shards (one expert per NeuronCore) with a
    fixed per-expert device capacity of 2048 tokens (the perfect-balance
    share). The few tokens past capacity (capacity-overflow spill) are
    computed on host in fp32 and added during unshard.
  - Device (per core): SwiGLU expert FFN over its gathered tokens,
    y = diag(scale) @ ((silu(x Wg^T) * (x Wu^T)) Wd^T), fp16 matmul
    operands with fp32 PSUM accumulation. Gate/up weights are loaded in
    16 f-sliced tiles in compute order so the first matmul only waits for
    x-tile0 + the first 256KB weight slice (~5us) instead of the full Wg.
  - Host: scatter-add per-expert outputs back into the [B,T,D] output.

Self-contained: hardcodes all shapes from the problem spec.
"""

import os
import numpy as np

# recover automatically if a prior run left the NeuronCores wedged
os.environ.setdefault("NEURON_RT_RESET_CORES", "1")

D = 1024
FF = 2048
E = 8
TOPK = 2
NCORES = 8
ND = D // 128    # 8 contraction chunks
NF = FF // 128   # 16 ff chunks
TT = 512         # token tile (moving-operand N per matmul)
CAP = 2048       # device tokens per expert; overflow spills to host

# matmul operand dtype on device ("float16", "bfloat16")
MM_DTYPE = os.environ.get("MOE_MM_DTYPE", "float16")

# test-only knobs / results (harness never touches these)
LAST_RESULTS = None
_NC_CACHE = {}


def split_multi_waits(nc, mybir_mod):
    """This walrus build rejects any instruction carrying more than one
    sync wait ("Too many sync wait commands"). Hoist extra waits onto
    single-wait NOPs inserted just before the instruction on the same
    engine — semantically identical since engines execute in order."""
    n_split = 0
    for f in nc.m.functions:
        for blk in f.blocks:
            insts = blk.instructions
            newl = []
            changed = False
            for inst in insts:
                si = inst.sync_info
                if si is not None and len(si.on_wait) > 1:
                    waits = list(si.on_wait)
                    del si.on_wait[1:]
                    for j, w in enumerate(waits[1:]):
                        nop = mybir_mod.InstNoOp(
                            name=f"{inst.name}_w{j}",
                            engine=inst.engine,
                            ins=[],
                            outs=[],
                        )
                        nop.sync_info = mybir_mod.SyncInfo(on_wait=[w], on_update=[])
                        newl.append(nop)
                        n_split += 1
                    changed = True
                newl.append(inst)
            if changed:
                insts[:] = newl
    return n_split


def build_nc(cap=CAP, repeat=1):
    """Build the per-core Bass program: SwiGLU FFN for one expert over
    `cap` tokens. Same NEFF on all 8 cores (SPMD).

    repeat>1 wraps the whole body (including weight loads) in a hardware
    loop — used only for benchmarking (dispatch overhead amortization)."""
    import contextlib

    import concourse.bass as bass
    import concourse.mybir as mybir
    import concourse.tile as tile

    dt = mybir.dt
    f32 = dt.float32
    mmdt = getattr(dt, MM_DTYPE)
    AF = mybir.ActivationFunctionType
    ng = cap // 128
    assert cap % TT == 0
    # first 512-token tile split in two 256s: the first matmul then only
    # waits for a 512KB x slice + one 256KB weight slice
    tiles = [(0, 256), (256, 256)]
    off = 256 + 256
    while off < cap:
        tiles.append((off, TT))
        off += TT

    nc = bass.Bass()
    # x^T arranged [128, d-chunk, token]; gate/up weights arranged
    # f-chunk-major so each [128, ND, 128] slice is one contiguous DMA
    xt = nc.dram_tensor("xt", [128, ND, cap], mmdt, kind="ExternalInput")
    wg = nc.dram_tensor("wg", [NF * 128, ND, 128], mmdt, kind="ExternalInput")
    wu = nc.dram_tensor("wu", [NF * 128, ND, 128], mmdt, kind="ExternalInput")
    wd = nc.dram_tensor("wd", [FF, D], mmdt, kind="ExternalInput")
    sc = nc.dram_tensor("sc", [128, ng], f32, kind="ExternalInput")
    # y stored at matmul-operand precision: the partial outputs are
    # combined on host in fp32; fp16 quantization here adds ~5e-4 rel
    # error against a 2e-2 budget and halves store bytes/tail latency
    y = nc.dram_tensor("y", [cap, D], mmdt, kind="ExternalOutput")

    with tile.TileContext(nc) as tc:
        with (
            tc.tile_pool(name="wpool", bufs=1) as wpool,
            tc.tile_pool(name="xpool", bufs=2) as xpool,
            tc.tile_pool(name="hpool", bufs=2) as hpool,
            tc.tile_pool(name="gpool", bufs=3) as gpool,
            tc.tile_pool(name="ypool", bufs=4) as ypool,
            tc.tile_pool(name="pg", bufs=2, space="PSUM") as pgpool,
            tc.tile_pool(name="pu", bufs=2, space="PSUM") as pupool,
            tc.tile_pool(name="po", bufs=4, space="PSUM") as popool,
        ):
            # PE warmup: a few matmuls on a zeroed scratch tile while the
            # first DMAs are in flight, so the HAM clock gate is already
            # at full rate when real matmuls start. Off the critical path
            # (PE would otherwise idle during the DMA lead-in). Outside the
            # benchmark repeat loop — only the first pass needs it.
            warm = wpool.tile([128, TT], mmdt, tag="warm")
            nc.gpsimd.memset(warm[:], 0)
            pwarm = pgpool.tile([128, TT], f32, tag="pg")
            for i in range(7):
                nc.tensor.matmul(pwarm[:], warm[:, 0:128], warm[:])
            rep_ctx = (
                tc.For_i(0, repeat, 1, hint_engines=(mybir.EngineType.PE,))
                if repeat > 1
                else contextlib.nullcontext()
            )
            rep_ctx.__enter__()
            # DMA issue order == compute-need order: x tile0, then
            # interleaved per-f gate/up weight slices, then down-proj
            # weights. Combine scales are first needed ~90us in.
            off0, tt0 = tiles[0]
            xt0 = xpool.tile([128, ND, tt0], mmdt, tag=f"xt{tt0}")
            nc.sync.dma_start(xt0[:], xt[:, :, off0 : off0 + tt0])
            wg_sb = []
            wu_sb = []
            s_sb = None
            for f in range(NF):
                tg = wpool.tile([128, ND, 128], mmdt, tag=f"wg{f}")
                nc.sync.dma_start(tg[:], wg[f * 128 : (f + 1) * 128, :, :])
                wg_sb.append(tg)
                tu = wpool.tile([128, ND, 128], mmdt, tag=f"wu{f}")
                nc.sync.dma_start(tu[:], wu[f * 128 : (f + 1) * 128, :, :])
                wu_sb.append(tu)
                if f == 0:
                    s_sb = wpool.tile([128, ng], f32, tag="s")
                    nc.sync.dma_start(s_sb[:], sc[:])
            wd_sb = []
            for f in range(NF):
                t = wpool.tile([128, D], mmdt, tag=f"wd{f}")
                nc.sync.dma_start(t[:], wd[f * 128 : (f + 1) * 128, :])
                wd_sb.append(t)

            for it, (off, tt) in enumerate(tiles):
                if it == 0:
                    xt_t = xt0
                else:
                    xt_t = xpool.tile([128, ND, tt], mmdt, tag=f"xt{tt}")
                    nc.sync.dma_start(xt_t[:], xt[:, :, off : off + tt])
                # gate/up + SwiGLU -> h^T [f, tokens]
                ht_t = []
                for f in range(NF):
                    pg = pgpool.tile([128, tt], f32, tag="pg")
                    pu = pupool.tile([128, tt], f32, tag="pu")
                    for j in range(ND):
                        nc.tensor.matmul(
                            pg[:],
                            wg_sb[f][:, j, :],
                            xt_t[:, j, :],
                            start=(j == 0),
                            stop=(j == ND - 1),
                        )
                    for j in range(ND):
                        nc.tensor.matmul(
                            pu[:],
                            wu_sb[f][:, j, :],
                            xt_t[:, j, :],
                            start=(j == 0),
                            stop=(j == ND - 1),
                        )
                    sg = gpool.tile([128, tt], mmdt, tag="sg")
                    nc.scalar.activation(sg[:], pg[:], AF.Silu)
                    ht = hpool.tile([128, tt], mmdt, tag=f"ht{f}")
                    nc.vector.tensor_mul(ht[:], sg[:], pu[:])
                    ht_t.append(ht)
                # down projection, scaled by combine weight per token
                for k in range(tt // 128):
                    g = off // 128 + k
                    if it == len(tiles) - 1 and k == tt // 128 - 1:
                        # final token group: chunk-major order (4 x 256-wide)
                        # so each chunk's copy+store overlaps the next
                        # chunk's matmuls — shortens the exposed kernel tail
                        for c in range(4):
                            po = popool.tile(
                                [128, 256], f32, tag="po", name=f"po_last_{c}"
                            )
                            for f in range(NF):
                                nc.tensor.matmul(
                                    po[:],
                                    ht_t[f][:, k * 128 : (k + 1) * 128],
                                    wd_sb[f][:, c * 256 : (c + 1) * 256],
                                    start=(f == 0),
                                    stop=(f == NF - 1),
                                )
                            yt = ypool.tile([128, 256], mmdt, tag="yt2")
                            if c % 2 == 0:
                                nc.scalar.activation(
                                    yt[:], po[:], AF.Copy, scale=s_sb[:, g : g + 1]
                                )
                            else:
                                nc.vector.tensor_scalar_mul(
                                    yt[:], po[:], s_sb[:, g : g + 1]
                                )
                            dge = nc.sync if c % 2 == 0 else nc.scalar
                            dge.dma_start(
                                y[
                                    off + k * 128 : off + (k + 1) * 128,
                                    c * 256 : (c + 1) * 256,
                                ],
                                yt[:],
                            )
                        continue
                    po_h = []
                    for dh in range(2):
                        po = popool.tile(
                            [128, 512], f32, tag="po", name=f"po_{it}_{k}_{dh}"
                        )
                        po_h.append(po)
                    for f in range(NF):
                        lhs = ht_t[f][:, k * 128 : (k + 1) * 128]
                        for dh in range(2):
                            nc.tensor.matmul(
                                po_h[dh][:],
                                lhs,
                                wd_sb[f][:, dh * 512 : (dh + 1) * 512],
                                start=(f == 0),
                                stop=(f == NF - 1),
                            )
                    # scale-by-combine-weight copies: dh=0 on ACT, dh=1 on
                    # DVE so the two run concurrently (shortens the kernel
                    # tail and halves ACT load)
                    for dh in range(2):
                        yt = ypool.tile([128, 512], mmdt, tag="yt")
                        if dh == 0:
                            nc.scalar.activation(
                                yt[:], po_h[dh][:], AF.Copy, scale=s_sb[:, g : g + 1]
                            )
                        else:
                            nc.vector.tensor_scalar_mul(
                                yt[:], po_h[dh][:], s_sb[:, g : g + 1]
                            )
                        # dh=1 stores go out on the Activation hwdge queue so
                        # the two stores of a group use parallel DGE queues
                        dge = nc.sync if dh == 0 else nc.scalar
                        dge.dma_start(
                            y[
                                off + k * 128 : off + (k + 1) * 128,
                                dh * 512 : (dh + 1) * 512,
                            ],
                            yt[:],
                        )
            rep_ctx.__exit__(None, None, None)
    split_multi_waits(nc, mybir)
    return nc


def _get_nc(cap=CAP):
    key = (cap, MM_DTYPE)
    if key not in _NC_CACHE:
        _NC_CACHE[key] = build_nc(cap)
    return _NC_CACHE[key]


def _route(xf, Wr):
    """fp32 softmax + top-2 + normalized combine weights, matching the
    jax reference (ties broken toward lower expert index)."""
    logits = xf @ Wr.astype(np.float32).T
    m = logits.max(-1, keepdims=True)
    ex = np.exp(logits - m)
    p = ex / ex.sum(-1, keepdims=True)
    top2 = np.argsort(-p, axis=-1, kind="stable")[:, :TOPK]
    n = xf.shape[0]
    p1 = p[np.arange(n), top2[:, 0]]
    p2 = p[np.arange(n), top2[:, 1]]
    denom = (p1 + p2) + np.float32(1e-8)
    return top2, p1 / denom, p2 / denom


def _prep_maps(inputs, cap=CAP):
    """Route + build per-core input maps. Returns
    (in_maps, dev_idxs, overflow, xf) where overflow is a list of
    (expert, token_idx_array, scale_array) for tokens past capacity."""
    x = np.asarray(inputs["x"])
    Wr = np.asarray(inputs["Wr"])
    Wg = np.asarray(inputs["Wg"])
    Wu = np.asarray(inputs["Wu"])
    Wd = np.asarray(inputs["Wd"])
    xf = x.reshape(-1, D).astype(np.float32, copy=False)

    top2, s1, s2 = _route(xf, Wr)

    mmnp = np.dtype(np.float16 if MM_DTYPE == "float16" else np.float32)
    if MM_DTYPE == "bfloat16":
        import ml_dtypes

        mmnp = np.dtype(ml_dtypes.bfloat16)
    xf_mm = xf.astype(mmnp)

    in_maps = []
    dev_idxs = []
    overflow = []
    for e in range(E):
        idx = np.nonzero((top2[:, 0] == e) | (top2[:, 1] == e))[0]
        sce = np.where(top2[idx, 0] == e, s1[idx], s2[idx]).astype(np.float32)
        n_dev = min(len(idx), cap)
        didx = idx[:n_dev]
        dev_idxs.append(didx)
        if n_dev < len(idx):
            overflow.append((e, idx[n_dev:], sce[n_dev:]))
        xt3 = np.zeros((128, ND, cap), dtype=mmnp)
        xt3[:, :, :n_dev] = xf_mm[didx].T.reshape(ND, 128, n_dev).transpose(1, 0, 2)
        scp = np.zeros(cap, dtype=np.float32)
        scp[:n_dev] = sce[:n_dev]
        wgT = Wg[e].T.reshape(ND, 128, NF, 128)
        wg4 = np.ascontiguousarray(
            wgT.transpose(2, 1, 0, 3).reshape(NF * 128, ND, 128)
        ).astype(mmnp)
        wuT = Wu[e].T.reshape(ND, 128, NF, 128)
        wu4 = np.ascontiguousarray(
            wuT.transpose(2, 1, 0, 3).reshape(NF * 128, ND, 128)
        ).astype(mmnp)
        in_maps.append(
            {
                "xt": xt3,
                "wg": wg4,
                "wu": wu4,
                "wd": np.ascontiguousarray(Wd[e].T).astype(mmnp),
                "sc": np.ascontiguousarray(scp.reshape(cap // 128, 128).T),
            }
        )
    return in_maps, dev_idxs, overflow, xf


def kernel(**inputs):
    global LAST_RESULTS
    from concourse.bass_utils import run_bass_kernel_spmd

    x = np.asarray(inputs["x"])
    B, T, _ = x.shape
    in_maps, dev_idxs, overflow, xf = _prep_maps(inputs)
    n_tok = xf.shape[0]

    nc = _get_nc(CAP)
    try:
        res = run_bass_kernel_spmd(nc, in_maps, list(range(NCORES)))
    except Exception:
        # A previously wedged NeuronCore fails the first execute attempt
        # (NRT_EXEC_UNIT_UNRECOVERABLE); resetting the PJRT backend and
        # retrying once recovers (cores reset via NEURON_RT_RESET_CORES).
        import jax
        import jax.extend as jex

        jax.clear_caches()
        try:
            jex.backend.clear_backends()
        except Exception:
            pass
        res = run_bass_kernel_spmd(nc, in_maps, list(range(NCORES)))
    LAST_RESULTS = res

    out = np.zeros((n_tok, D), dtype=np.float32)
    for e in range(E):
        didx = dev_idxs[e]
        out[didx] += res.results[e]["y"][: len(didx)]
    # host-side capacity-overflow spill (fp32, exact)
    if overflow:
        Wg = np.asarray(inputs["Wg"], dtype=np.float32)
        Wu = np.asarray(inputs["Wu"], dtype=np.float32)
        Wd = np.asarray(inputs["Wd"], dtype=np.float32)
        for e, oidx, osc in overflow:
            xo = xf[oidx]
            g = xo @ Wg[e].T
            u = xo @ Wu[e].T
            h = (g / (1.0 + np.exp(-g))) * u
            out[oidx] += osc[:, None] * (h @ Wd[e].T)
    return out.reshape(B, T, D).astype(x.dtype, copy=False)



# revision 6
# speedup vs baseline: 1.2458x; 1.2458x over previous
"""Trainium2 Bass kernel for top-2 MoE (nn_MoE_2113123910117).

Strategy (expert-parallel + combine-weight-aware mixed precision):
  - Host: router softmax -> top-2 -> normalized combine weights. Every
    top-1 (token, expert) pair and every top-2 pair with combine weight
    s2 >= TAU runs on device in fp16. Top-2 pairs with s2 < TAU (~52% of
    them) run on device in fp8e4m3 using DoubleRow matmuls (2x PE MAC
    rate); their output error (~6.6% rel) is attenuated by s2 < 0.38,
    keeping the global rel error ~1.6e-2 (< 2e-2 budget).
  - Cores are grouped in pairs; each pair owns two experts. Per core the
    NEFF runs 4 sub-batches: A1/A2 = fp16 FFN over 768 tokens for expert
    i/j, B1/B2 = fp8 FFN over 256 tokens for expert i/j. Each expert's
    fp16 tokens split across its pair's two A-slots (2x768 = 1536 cap),
    fp8 tokens across the B-slots (512 cap). Capacity overflow (~1.5% of
    pairs, chosen as the largest-s2 fp8 tokens) is computed on host in
    fp32 and added during unshard.
  - fp16 path: y = diag(s) ((silu(x Wg^T) * (x Wu^T)) Wd^T), fp16 matmul
    operands, fp32 PSUM.
  - fp8 path: weights pre-scaled by 8 and quantized to fp8e4m3; x
    quantized to fp8e4m3. DoubleRow matmuls with K=256 per instruction.
    silu applied with scale 1/8 on ACT (fp16 out), h quantized to fp8 by
    the DVE multiply, down-projection epilogue scale s/64 folds out the
    weight prescaling.

Self-contained: hardcodes all shapes from the problem spec.
"""

import os
import numpy as np

# recover automatically if a prior run left the NeuronCores wedged
os.environ.setdefault("NEURON_RT_RESET_CORES", "1")

D = 1024
FF = 2048
E = 8
TOPK = 2
NCORES = 8
ND = D // 128    # 8 contraction chunks of 128
NF = FF // 128   # 16 ff chunks of 128
NKP = ND // 2    # 4 k-pairs (256-deep DoubleRow contraction) for gate/up
NFP = NF // 2    # 8 k-pairs for the fp8 down projection
CAP16 = 1536     # fp16 tokens per core (2 sub-batches of 768)
CAP8 = 512       # fp8 tokens per core (2 sub-batches of 256)
ATT = 256        # fp16 token tile
TAU = 0.38       # top-2 combine-weight threshold for the fp8 path
SW = 8.0         # fp8 weight prescale; down epilogue folds 1/SW^2

# test-only knobs / results (harness never touches these)
LAST_RESULTS = None
_NC_CACHE = {}


def split_multi_waits(nc, mybir_mod):
    """This walrus build rejects any instruction carrying more than one
    sync wait ("Too many sync wait commands"). Hoist extra waits onto
    single-wait NOPs inserted just before the instruction on the same
    engine — semantically identical since engines execute in order."""
    n_split = 0
    for f in nc.m.functions:
        for blk in f.blocks:
            insts = blk.instructions
            newl = []
            changed = False
            for inst in insts:
                si = inst.sync_info
                if si is not None and len(si.on_wait) > 1:
                    waits = list(si.on_wait)
                    del si.on_wait[1:]
                    for j, w in enumerate(waits[1:]):
                        nop = mybir_mod.InstNoOp(
                            name=f"{inst.name}_w{j}",
                            engine=inst.engine,
                            ins=[],
                            outs=[],
                        )
                        nop.sync_info = mybir_mod.SyncInfo(on_wait=[w], on_update=[])
                        newl.append(nop)
                        n_split += 1
                    changed = True
                newl.append(inst)
            if changed:
                insts[:] = newl
    return n_split


def build_nc(caps=(CAP16, CAP8), repeat=1):
    """Build the per-core Bass program: two fp16 expert FFN sub-batches
    followed by two fp8 (DoubleRow) sub-batches. Same NEFF on all 8
    cores (SPMD); the expert identity lives entirely in the input maps.

    repeat>1 wraps the whole body (including weight loads) in a hardware
    loop — used only for benchmarking (dispatch overhead amortization)."""
    import contextlib

    import concourse.bass as bass
    import concourse.mybir as mybir
    import concourse.tile as tile

    dt = mybir.dt
    f32 = dt.float32
    f16 = dt.float16
    e4 = dt.float8e4
    AF = mybir.ActivationFunctionType
    DR = mybir.MatmulPerfMode.DoubleRow

    cap16, cap8 = caps
    a_sub = cap16 // 2
    b_sub = cap8 // 2
    na_t = a_sub // ATT       # fp16 token tiles per sub-batch
    ng16 = cap16 // 128       # combine-scale groups, fp16
    ng8 = cap8 // 128

    nc = bass.Bass()
    # fp16 tokens: x^T arranged [128, d-chunk, token]
    xt = nc.dram_tensor("xt", [128, ND, cap16], f16, kind="ExternalInput")
    # fp8 tokens, same layout
    x8 = nc.dram_tensor("x8", [128, ND, cap8], e4, kind="ExternalInput")
    # fp16 weights, one set per expert of the pair; gate/up f-chunk-major
    wg = nc.dram_tensor("wg", [2, NF * 128, ND, 128], f16, kind="ExternalInput")
    wu = nc.dram_tensor("wu", [2, NF * 128, ND, 128], f16, kind="ExternalInput")
    wd = nc.dram_tensor("wd", [2, FF, D], f16, kind="ExternalInput")
    # fp8 weights (pre-scaled by SW): [set, chunk*128(+p), kpair, ktile, feat]
    wg8 = nc.dram_tensor("wg8", [2, NF * 128, NKP, 2, 128], e4, kind="ExternalInput")
    wu8 = nc.dram_tensor("wu8", [2, NF * 128, NKP, 2, 128], e4, kind="ExternalInput")
    # fp8 down weights: [set, p(f within chunk), kpair, ktile, d]
    wd8 = nc.dram_tensor("wd8", [2, 128, NFP, 2, D], e4, kind="ExternalInput")
    sc = nc.dram_tensor("sc", [128, ng16], f32, kind="ExternalInput")
    sc8 = nc.dram_tensor("sc8", [128, ng8], f32, kind="ExternalInput")
    # outputs at fp16: partial rows are combined on host in fp32
    y = nc.dram_tensor("y", [cap16, D], f16, kind="ExternalOutput")
    y8 = nc.dram_tensor("y8", [cap8, D], f16, kind="ExternalOutput")

    with tile.TileContext(nc) as tc:
        with (
            tc.tile_pool(name="wpool", bufs=1) as wpool,
            tc.tile_pool(name="w8pool", bufs=1) as w8pool,
            tc.tile_pool(name="xpool", bufs=1) as xpool,
            tc.tile_pool(name="hpool", bufs=2) as hpool,
            tc.tile_pool(name="gpool", bufs=3) as gpool,
            tc.tile_pool(name="ypool", bufs=4) as ypool,
            tc.tile_pool(name="pg", bufs=2, space="PSUM") as pgpool,
            tc.tile_pool(name="pu", bufs=2, space="PSUM") as pupool,
            tc.tile_pool(name="po", bufs=4, space="PSUM") as popool,
        ):
            # PE warmup: matmuls on a zeroed scratch tile while the first
            # DMAs are in flight, so the p-state clock gate is already
            # ramping when real matmuls start. Outside the repeat loop.
            warm = wpool.tile([128, ATT], f16, tag="warm")
            nc.gpsimd.memset(warm[:], 0)
            pwarm = pgpool.tile([128, ATT], f32, tag="pg")
            for i in range(7):
                nc.tensor.matmul(pwarm[:], warm[:, 0:128], warm[:])
            rep_ctx = (
                tc.For_i(0, repeat, 1, hint_engines=(mybir.EngineType.PE,))
                if repeat > 1
                else contextlib.nullcontext()
            )
            rep_ctx.__enter__()

            # Resident token tensors; slices feed the matmuls directly.
            xt_sb = xpool.tile([128, ND, cap16], f16, tag="xt")
            x8_sb = xpool.tile([128, ND, cap8], e4, tag="x8")
            s_sb = wpool.tile([128, ng16], f32, tag="s")
            s8_sb = wpool.tile([128, ng8], f32, tag="s8")

            # ---------------- phase A: fp16, two sub-batches ----------------
            for s in range(2):
                # DMA issue order == compute-need order. First sub-batch:
                # token tiles interleaved with the first weight slices so
                # the first matmul waits only ~1MB of DMA.
                wg_sb = [None] * NF
                wu_sb = [None] * NF
                for f in range(NF):
                    # interleave the first token-tile DMAs with the first
                    # weight slices so compute never waits on the queue
                    if f < na_t:
                        off = s * a_sub + f * ATT
                        nc.sync.dma_start(
                            xt_sb[:, :, off : off + ATT], xt[:, :, off : off + ATT]
                        )
                    tg = wpool.tile([128, ND, 128], f16, tag=f"wg{f}")
                    nc.sync.dma_start(tg[:], wg[s, f * 128 : (f + 1) * 128])
                    wg_sb[f] = tg
                    tu = wpool.tile([128, ND, 128], f16, tag=f"wu{f}")
                    nc.sync.dma_start(tu[:], wu[s, f * 128 : (f + 1) * 128])
                    wu_sb[f] = tu
                    if s == 0 and f == 0:
                        nc.sync.dma_start(s_sb[:], sc[:])
                wd_sb = []
                for f in range(NF):
                    t_ = wpool.tile([128, D], f16, tag=f"wd{f}")
                    nc.sync.dma_start(t_[:], wd[s, f * 128 : (f + 1) * 128, :])
                    wd_sb.append(t_)

                for t in range(na_t):
                    off = s * a_sub + t * ATT
                    # gate/up + SwiGLU -> h^T [f, tokens]
                    ht_t = []
                    for f in range(NF):
                        pg = pgpool.tile([128, ATT], f32, tag="pg")
                        pu = pupool.tile([128, ATT], f32, tag="pu")
                        for j in range(ND):
                            nc.tensor.matmul(
                                pg[:],
                                wg_sb[f][:, j, :],
                                xt_sb[:, j, off : off + ATT],
                                start=(j == 0),
                                stop=(j == ND - 1),
                            )
                        for j in range(ND):
                            nc.tensor.matmul(
                                pu[:],
                                wu_sb[f][:, j, :],
                                xt_sb[:, j, off : off + ATT],
                                start=(j == 0),
                                stop=(j == ND - 1),
                            )
                        sg = gpool.tile([128, ATT], f16, tag="sg")
                        nc.scalar.activation(sg[:], pg[:], AF.Silu)
                        ht = hpool.tile([128, ATT], f16, tag=f"ht{f}")
                        nc.vector.tensor_mul(ht[:], sg[:], pu[:])
                        ht_t.append(ht)
                    # down projection, scaled by combine weight per token
                    for k in range(ATT // 128):
                        g = off // 128 + k
                        po_h = []
                        for dh in range(2):
                            po = popool.tile(
                                [128, 512], f32, tag="po", name=f"po_{s}_{t}_{k}_{dh}"
                            )
                            po_h.append(po)
                        for f in range(NF):
                            lhs = ht_t[f][:, k * 128 : (k + 1) * 128]
                            for dh in range(2):
                                nc.tensor.matmul(
                                    po_h[dh][:],
                                    lhs,
                                    wd_sb[f][:, dh * 512 : (dh + 1) * 512],
                                    start=(f == 0),
                                    stop=(f == NF - 1),
                                )
                        # scale-by-combine-weight copies: dh=0 on ACT, dh=1
                        # on DVE so the two run concurrently
                        for dh in range(2):
                            yt = ypool.tile([128, 512], f16, tag="yt")
                            if dh == 0:
                                nc.scalar.activation(
                                    yt[:], po_h[dh][:], AF.Copy,
                                    scale=s_sb[:, g : g + 1],
                                )
                            else:
                                nc.vector.tensor_scalar_mul(
                                    yt[:], po_h[dh][:], s_sb[:, g : g + 1]
                                )
                            dge = nc.sync if dh == 0 else nc.scalar
                            dge.dma_start(
                                y[
                                    g * 128 : (g + 1) * 128,
                                    dh * 512 : (dh + 1) * 512,
                                ],
                                yt[:],
                            )

            # ---------------- phase B: fp8 DoubleRow, two sub-batches -------
            # weight DMAs for set 0 queue behind phase A's loads and run
            # during phase A compute.
            wg8_sb = {}
            wu8_sb = {}
            wd8_sb = {}
            for s in range(2):
                if s == 0:
                    nc.sync.dma_start(x8_sb[:], x8[:])
                    nc.sync.dma_start(s8_sb[:], sc8[:])
                for c in range(NF):
                    t8g = w8pool.tile([128, NKP, 2, 128], e4, tag=f"wg8_{c}")
                    nc.sync.dma_start(t8g[:], wg8[s, c * 128 : (c + 1) * 128])
                    wg8_sb[c] = t8g
                    t8u = w8pool.tile([128, NKP, 2, 128], e4, tag=f"wu8_{c}")
                    nc.sync.dma_start(t8u[:], wu8[s, c * 128 : (c + 1) * 128])
                    wu8_sb[c] = t8u
                t8d = w8pool.tile([128, NFP, 2, D], e4, tag="wd8")
                nc.sync.dma_start(t8d[:], wd8[s])
                wd8_sb[0] = t8d

                boff = s * b_sub
                ht8 = {}
                for c in range(NF):
                    pg = pgpool.tile([128, ATT], f32, tag="pg")
                    pu = pupool.tile([128, ATT], f32, tag="pu")
                    for j in range(NKP):
                        nc.tensor.matmul(
                            pg[:],
                            wg8_sb[c][:, j, :, :],
                            x8_sb[:, 2 * j : 2 * j + 2, boff : boff + b_sub],
                            start=(j == 0),
                            stop=(j == NKP - 1),
                            perf_mode=DR,
                        )
                    for j in range(NKP):
                        nc.tensor.matmul(
                            pu[:],
                            wu8_sb[c][:, j, :, :],
                            x8_sb[:, 2 * j : 2 * j + 2, boff : boff + b_sub],
                            start=(j == 0),
                            stop=(j == NKP - 1),
                            perf_mode=DR,
                        )
                    sg = gpool.tile([128, ATT], f16, tag="sg")
                    nc.scalar.activation(sg[:], pg[:], AF.Silu, scale=1.0 / SW)
                    if c % 2 == 0:
                        hj = hpool.tile(
                            [128, 2, b_sub], e4, tag=f"ht8_{c // 2}"
                        )
                        ht8[c // 2] = hj
                    nc.vector.tensor_mul(ht8[c // 2][:, c % 2, :], sg[:], pu[:])
                # down projection: token groups of 128, d-quarters of 256
                for k in range(b_sub // 128):
                    g = s * (b_sub // 128) + k
                    for dq in range(4):
                        po = popool.tile(
                            [128, 512], f32, tag="po", name=f"po8_{s}_{k}_{dq}"
                        )
                        for j in range(NFP):
                            nc.tensor.matmul(
                                po[:, 0:256],
                                ht8[j][:, :, k * 128 : (k + 1) * 128],
                                wd8_sb[0][:, j, :, dq * 256 : (dq + 1) * 256],
                                start=(j == 0),
                                stop=(j == NFP - 1),
                                perf_mode=DR,
                            )
                        yt = ypool.tile([128, 256], f16, tag="yt8")
                        if dq % 2 == 0:
                            nc.scalar.activation(
                                yt[:], po[:, 0:256], AF.Copy,
                                scale=s8_sb[:, g : g + 1],
                            )
                        else:
                            nc.vector.tensor_scalar_mul(
                                yt[:], po[:, 0:256], s8_sb[:, g : g + 1]
                            )
                        dge = nc.sync if dq % 2 == 0 else nc.scalar
                        dge.dma_start(
                            y8[
                                g * 128 : (g + 1) * 128,
                                dq * 256 : (dq + 1) * 256,
                            ],
                            yt[:],
                        )
            rep_ctx.__exit__(None, None, None)
    split_multi_waits(nc, mybir)
    return nc


def _get_nc(caps=(CAP16, CAP8)):
    key = caps
    if key not in _NC_CACHE:
        _NC_CACHE[key] = build_nc(caps)
    return _NC_CACHE[key]


def _route(xf, Wr):
    """fp32 softmax + top-2 + normalized combine weights, matching the
    jax reference (ties broken toward lower expert index)."""
    logits = xf @ Wr.astype(np.float32).T
    m = logits.max(-1, keepdims=True)
    ex = np.exp(logits - m)
    p = ex / ex.sum(-1, keepdims=True)
    top2 = np.argsort(-p, axis=-1, kind="stable")[:, :TOPK]
    n = xf.shape[0]
    p1 = p[np.arange(n), top2[:, 0]]
    p2 = p[np.arange(n), top2[:, 1]]
    denom = (p1 + p2) + np.float32(1e-8)
    return top2, p1 / denom, p2 / denom


def _pack_w16(W, mmnp):
    """Gate/up fp16 weight packing: [NF*128, ND, 128] f-chunk-major."""
    WT = W.T.reshape(ND, 128, NF, 128)
    return np.ascontiguousarray(
        WT.transpose(2, 1, 0, 3).reshape(NF * 128, ND, 128)
    ).astype(mmnp)


def _pack_w8_gu(W, e4np):
    """fp8 gate/up packing: [NF*128, NKP, 2, 128] where
    [c*128+p, j, i, m] = SW*W[c*128+m, (2j+i)*128+p]."""
    t = (W * SW).reshape(NF, 128, NKP, 2, 128)  # [c, m, j, i, p]
    return np.ascontiguousarray(t.transpose(0, 4, 2, 3, 1)).reshape(
        NF * 128, NKP, 2, 128
    ).astype(e4np)


def _pack_w8_d(Wde, e4np):
    """fp8 down packing: [128, NFP, 2, D] where
    [p, j, i, d] = SW*Wd[d, (2j+i)*128+p]."""
    t = (Wde * SW).reshape(D, NFP, 2, 128)  # [d, j, i, p]
    return np.ascontiguousarray(t.transpose(3, 1, 2, 0)).astype(e4np)


def _prep_maps(inputs, caps=(CAP16, CAP8)):
    """Route + build per-core input maps. Returns
    (in_maps, seg16, seg8, overflow, xf) where seg16/seg8 map core ->
    token index array for the y/y8 outputs and overflow is a list of
    (expert, token_idx_array, scale_array) computed on host."""
    import ml_dtypes

    cap16, cap8 = caps
    a_sub = cap16 // 2
    b_sub = cap8 // 2
    x = np.asarray(inputs["x"])
    Wr = np.asarray(inputs["Wr"])
    Wg = np.asarray(inputs["Wg"])
    Wu = np.asarray(inputs["Wu"])
    Wd = np.asarray(inputs["Wd"])
    xf = x.reshape(-1, D).astype(np.float32, copy=False)

    top2, s1, s2 = _route(xf, Wr)

    f16np = np.dtype(np.float16)
    e4np = np.dtype(ml_dtypes.float8_e4m3)
    xf_16 = xf.astype(f16np)
    xf_8 = xf.astype(e4np)

    # per-expert token lists
    idx16_e, sc16_e, idx8_e, sc8_e = [], [], [], []
    overflow = []
    n = xf.shape[0]
    for e in range(E):
        i_top1 = np.nonzero(top2[:, 0] == e)[0]
        s_top1 = s1[i_top1]
        i_top2 = np.nonzero(top2[:, 1] == e)[0]
        s_top2 = s2[i_top2]
        hi = s_top2 >= TAU
        i16 = np.concatenate([i_top1, i_top2[hi]])
        sc16v = np.concatenate([s_top1, s_top2[hi]]).astype(np.float32)
        # fp8 candidates sorted ascending s2 so capacity spill takes the
        # largest-s2 tokens (computed exactly on host -> least fp8 error)
        i8 = i_top2[~hi]
        s8v = s_top2[~hi].astype(np.float32)
        o = np.argsort(s8v, kind="stable")
        i8, s8v = i8[o], s8v[o]
        if len(i16) > cap16:
            overflow.append((e, i16[cap16:], sc16v[cap16:]))
            i16, sc16v = i16[:cap16], sc16v[:cap16]
        if len(i8) > cap8:
            overflow.append((e, i8[cap8:], s8v[cap8:]))
            i8, s8v = i8[:cap8], s8v[:cap8]
        idx16_e.append(i16)
        sc16_e.append(sc16v)
        idx8_e.append(i8)
        sc8_e.append(s8v)

    # pair experts to balance per-core cycles (384/pair fp16, 192 fp8)
    cyc = [384 * len(idx16_e[e]) + 192 * len(idx8_e[e]) for e in range(E)]
    order = np.argsort(np.asarray(cyc))
    pairs = [(int(order[7 - k]), int(order[k])) for k in range(E // 2)]

    in_maps = [None] * NCORES
    seg16 = [None] * NCORES
    seg8 = [None] * NCORES
    for k, (ea, eb) in enumerate(pairs):
        wgs = np.stack([_pack_w16(Wg[ea], f16np), _pack_w16(Wg[eb], f16np)])
        wus = np.stack([_pack_w16(Wu[ea], f16np), _pack_w16(Wu[eb], f16np)])
        wds = np.stack(
            [
                np.ascontiguousarray(Wd[ea].T).astype(f16np),
                np.ascontiguousarray(Wd[eb].T).astype(f16np),
            ]
        )
        wg8s = np.stack([_pack_w8_gu(Wg[ea], e4np), _pack_w8_gu(Wg[eb], e4np)])
        wu8s = np.stack([_pack_w8_gu(Wu[ea], e4np), _pack_w8_gu(Wu[eb], e4np)])
        wd8s = np.stack([_pack_w8_d(Wd[ea], e4np), _pack_w8_d(Wd[eb], e4np)])
        for half in range(2):
            core = 2 * k + half
            xt3 = np.zeros((128, ND, cap16), dtype=f16np)
            x83 = np.zeros((128, ND, cap8), dtype=e4np)
            scp = np.zeros(cap16, dtype=np.float32)
            sc8p = np.zeros(cap8, dtype=np.float32)
            t16 = []
            t8 = []
            for s, e_ in enumerate((ea, eb)):
                i16, v16 = idx16_e[e_], sc16_e[e_]
                nhalf = (len(i16) + 1) // 2
                seg = i16[:nhalf] if half == 0 else i16[nhalf:]
                vseg = v16[:nhalf] if half == 0 else v16[nhalf:]
                assert len(seg) <= a_sub
                off = s * a_sub
                xt3[:, :, off : off + len(seg)] = (
                    xf_16[seg].T.reshape(ND, 128, len(seg)).transpose(1, 0, 2)
                )
                scp[off : off + len(seg)] = vseg
                t16.append(seg)
                i8, v8 = idx8_e[e_], sc8_e[e_]
                nhalf8 = (len(i8) + 1) // 2
                seg8_ = i8[:nhalf8] if half == 0 else i8[nhalf8:]
                vseg8 = v8[:nhalf8] if half == 0 else v8[nhalf8:]
                assert len(seg8_) <= b_sub
                off8 = s * b_sub
                x83[:, :, off8 : off8 + len(seg8_)] = (
                    xf_8[seg8_]
                    .astype(e4np)
                    .T.reshape(ND, 128, len(seg8_))
                    .transpose(1, 0, 2)
                )
                sc8p[off8 : off8 + len(seg8_)] = vseg8 / np.float32(SW * SW)
                t8.append(seg8_)
            in_maps[core] = {
                "xt": xt3,
                "x8": x83,
                "wg": wgs,
                "wu": wus,
                "wd": wds,
                "wg8": wg8s,
                "wu8": wu8s,
                "wd8": wd8s,
                "sc": np.ascontiguousarray(
                    scp.reshape(cap16 // 128, 128).T
                ),
                "sc8": np.ascontiguousarray(
                    sc8p.reshape(cap8 // 128, 128).T
                ),
            }
            seg16[core] = t16
            seg8[core] = t8
    return in_maps, seg16, seg8, overflow, xf


def kernel(**inputs):
    global LAST_RESULTS
    from concourse.bass_utils import run_bass_kernel_spmd

    x = np.asarray(inputs["x"])
    B, T, _ = x.shape
    caps = (CAP16, CAP8)
    in_maps, seg16, seg8, overflow, xf = _prep_maps(inputs, caps)
    n_tok = xf.shape[0]
    a_sub, b_sub = caps[0] // 2, caps[1] // 2

    nc = _get_nc(caps)
    try:
        res = run_bass_kernel_spmd(nc, in_maps, list(range(NCORES)))
    except Exception:
        # A previously wedged NeuronCore fails the first execute attempt
        # (NRT_EXEC_UNIT_UNRECOVERABLE); resetting the PJRT backend and
        # retrying once recovers (cores reset via NEURON_RT_RESET_CORES).
        import jax
        import jax.extend as jex

        jax.clear_caches()
        try:
            jex.backend.clear_backends()
        except Exception:
            pass
        res = run_bass_kernel_spmd(nc, in_maps, list(range(NCORES)))
    LAST_RESULTS = res

    out = np.zeros((n_tok, D), dtype=np.float32)
    for core in range(NCORES):
        y16 = np.asarray(res.results[core]["y"]).astype(np.float32)
        y8v = np.asarray(res.results[core]["y8"]).astype(np.float32)
        for s in range(2):
            seg = seg16[core][s]
            out[seg] += y16[s * a_sub : s * a_sub + len(seg)]
            sg8 = seg8[core][s]
            out[sg8] += y8v[s * b_sub : s * b_sub + len(sg8)]
    # host-side capacity-overflow spill (fp32, exact)
    if overflow:
        Wg = np.asarray(inputs["Wg"], dtype=np.float32)
        Wu = np.asarray(inputs["Wu"], dtype=np.float32)
        Wd = np.asarray(inputs["Wd"], dtype=np.float32)
        for e, oidx, osc in overflow:
            xo = xf[oidx]
            g = xo @ Wg[e].T
            u = xo @ Wu[e].T
            h = (g / (1.0 + np.exp(-g))) * u
            out[oidx] += osc[:, None] * (h @ Wd[e].T)
    return out.reshape(B, T, D).astype(x.dtype, copy=False)
